# revision 21
# baseline (speedup 1.0000x reference)
"""Trainium2 Bass kernel for nn_Actor (4D strided Minkowski-style conv net + MLP head).

v3: instruction-count overhaul of the z-block-sharded scheme (Z=96 -> 8 blocks
of 12; core i owns conv2-z slice i, conv4 partials AllReduced, 4KB f32).

Key structural choices (driven by trace analysis of v2 @207us):
- Pool = ONE nc.vector.tensor_reduce per chunk (slab-last host layout), not a
  3-op DVE tree. xp2 (K rows 128..161) packs 2 chunks at partition bases 0 and
  64 of one [128, 3200] tile so ONE reduce covers both chunks (DVE cost is
  free-dim-bound; rows 34..63 / 98..127 are garbage and never read). The
  chunk-b conv1 matmul runs at partition base 64 (legal for K<=64) against a
  second copy of w1b parked at partitions 64..97.
- conv1: PC=400 cols/chunk, 16 chunks, 2 chunks per 1.64MB DMA.
- conv2: tap-pairs (kz 0/1) packed into K=128 stationaries. h1 is written
  twice: rows 0..63 at col offset +4, rows 64..127 at offset 0, so one
  rectangular rhs view reads tap kz=0 from the lower half and kz=1 from the
  upper half. 8 matmuls per stripe-batch, 3 batches (stripes 0-4, 5-7, 8-9).
- conv4 flipped: stationary = h2[:, 4q:4q+4] (K=128 ch, M=4 batch), moving =
  w4 slice [128, 256 och] -> out [4, 256] PSUM accumulated over q. 100
  LDW+MM pairs with 256-col moving operands (vs 200 pairs of 4-col MMs).
- MLP head identical to v2; h256 [4,256] transposed to 2x[128,4] via
  matmul-with-eye4 before the BN bias/relu.

All heavy tensors bf16 (fp32 PSUM accumulation), BN folded into conv/linear
weights host-side. Host prep does layout/precision only.
"""

import sys

sys.path.insert(0, "/opt/trn_rl_repo")

from contextlib import ExitStack

import ml_dtypes
import numpy as np

import concourse.bass as bass
import concourse.tile as tile
from concourse import bacc, mybir
from concourse.bass_utils import run_bass_kernel_spmd

EPS = 1e-5
F32 = mybir.dt.float32
BF16 = mybir.dt.bfloat16
AF = mybir.ActivationFunctionType
ALU = mybir.AluOpType
BF = ml_dtypes.bfloat16

PC = 400          # patch columns per chunk
NCH = 16          # chunks
NG = 8            # DMA/pool groups (2 chunks each)
# conv2/conv4 batches: (after group, stripe range, q range)
BATCHES = [(3, 0, 5, 0, 50), (6, 5, 8, 50, 80), (7, 8, 10, 80, 100)]

LAST_EXEC_NS = None
_CACHE = {}


def _build():
    nc = bacc.Bacc(
        "TRN2",
        target_bir_lowering=False,
        debug=False,
        enable_asserts=False,
        num_devices=8,
    )
    d = {}

    def din(name, shape, dt=BF16):
        d[name] = nc.dram_tensor(name, list(shape), dt, kind="ExternalInput").ap()
        return d[name]

    xp1 = din("xp1", (128, NG, 6400))
    xp2 = din("xp2", (NG, 68, 3200))
    w1a = din("w1a", (128, 128))
    w1b = din("w1b", (34, 128))
    b1 = din("b1", (128, 1), F32)
    w2p = din("w2p", (128, 8 * 128))
    b2 = din("b2", (128, 1), F32)
    w4 = din("w4", (128, 100 * 256))
    b4 = din("b4", (128, 2), F32)
    eye4 = din("eye4", (4, 4), F32)
    wl0 = din("wl0", (12, 512), F32)
    wl1 = din("wl1", (128, 512), F32)
    wl2 = din("wl2", (128, 512), F32)
    bl = din("bl", (128, 4), F32)
    wo = din("wo", (128, 24), F32)
    bo = din("bo", (6, 1), F32)
    jnt = din("jnt", (12, 4), F32)
    out = nc.dram_tensor("out", [6, 4], F32, kind="ExternalOutput").ap()

    with TileKernel(nc) as tk:
        tk.run(xp1, xp2, w1a, w1b, b1, w2p, b2, w4, b4, eye4,
               wl0, wl1, wl2, bl, wo, bo, jnt, out)
    nc.compile()
    return nc


class TileKernel:
    def __init__(self, nc):
        self.nc = nc
        self.tc = tile.TileContext(nc)
        self.ctx = ExitStack()

    def __enter__(self):
        self.tc.__enter__()
        return self

    def __exit__(self, *a):
        self.ctx.close()
        return self.tc.__exit__(*a)

    def run(self, xp1, xp2, w1a, w1b, b1, w2p, b2, w4, b4, eye4,
            wl0, wl1, wl2, bl, wo, bo, jnt, out):
        nc, tc, ctx = self.nc, self.tc, self.ctx
        sync = nc.sync

        const = ctx.enter_context(tc.tile_pool(name="const", bufs=1))
        stream1 = ctx.enter_context(tc.tile_pool(name="stream1", bufs=3))
        stream2 = ctx.enter_context(tc.tile_pool(name="stream2", bufs=3))
        pools = ctx.enter_context(tc.tile_pool(name="pools", bufs=4))
        big = ctx.enter_context(tc.tile_pool(name="big", bufs=1))
        ps1 = ctx.enter_context(tc.tile_pool(name="ps1", bufs=2, space="PSUM"))
        ps2 = ctx.enter_context(tc.tile_pool(name="ps2", bufs=1, space="PSUM"))
        ps4 = ctx.enter_context(tc.tile_pool(name="ps4", bufs=1, space="PSUM"))
        pst = ctx.enter_context(tc.tile_pool(name="pst", bufs=1, space="PSUM"))
        dram = ctx.enter_context(tc.tile_pool(name="dram", bufs=1, space="DRAM"))

        # ---- constants into SBUF ----
        w1a_t = const.tile([128, 128], BF16); sync.dma_start(w1a_t[:], w1a)
        w1b_t = const.tile([34, 128], BF16); sync.dma_start(w1b_t[:], w1b)
        w1bb_t = const.tile([98, 128], BF16)
        sync.dma_start(w1bb_t[64:98, :], w1b)
        b1_t = const.tile([128, 1], F32); sync.dma_start(b1_t[:], b1)
        w2p_t = const.tile([128, 8 * 128], BF16); sync.dma_start(w2p_t[:], w2p)
        b2_t = const.tile([128, 1], F32); sync.dma_start(b2_t[:], b2)
        b4_t = const.tile([128, 2], F32); sync.dma_start(b4_t[:], b4)
        eye_t = const.tile([4, 4], F32); sync.dma_start(eye_t[:], eye4)
        wl0_t = const.tile([12, 512], F32); sync.dma_start(wl0_t[:], wl0)
        wl1_t = const.tile([128, 512], F32); sync.dma_start(wl1_t[:], wl1)
        wl2_t = const.tile([128, 512], F32); sync.dma_start(wl2_t[:], wl2)
        bl_t = const.tile([128, 4], F32); sync.dma_start(bl_t[:], bl)
        wo_t = const.tile([128, 24], F32); sync.dma_start(wo_t[:], wo)
        bo_t = const.tile([6, 1], F32); sync.dma_start(bo_t[:], bo)
        jnt_t = const.tile([12, 4], F32); sync.dma_start(jnt_t[:], jnt)

        # w4 prefetched in 5 stripes of [128, 5120] on the scalar ring
        w4_t = const.tile([128, 100 * 256], BF16)
        SW = 20 * 256

        def w4_fetch(s):
            nc.scalar.dma_start(w4_t[:, s * SW:(s + 1) * SW],
                                w4[:, s * SW:(s + 1) * SW])

        w4_fetch(0)
        w4v = w4_t[:].rearrange("p (q o) -> p q o", q=100)

        # h1: [128, 6404] bf16. rows 0..63 hold data at tile col = data+4,
        # rows 64..127 hold the same data at tile col = data. One rectangular
        # conv2 rhs view then pairs tap kz=0 (lower) with kz=1 (upper).
        h1 = big.tile([128, 6404], BF16)
        # data-col view (tile cols 4..6403 on the lower half's coordinates)
        h1v = h1[:, 4:6404].rearrange(
            "p (qx qy pt lx ly lz b) -> p qx qy pt lx ly lz b",
            qx=10, qy=10, pt=2, lx=2, ly=2, lz=2)
        h2 = big.tile([128, 400], BF16)       # col = q*4 + b
        p4 = ps4.tile([4, 256], F32)          # conv4 acc: [batch, och]

        w4_fetched = 1

        # ---- streaming phase ----
        for g in range(NG):
            t1 = stream1.tile([128, 6400], BF16, tag="t1")
            sync.dma_start(t1[:], xp1[:, g, :])
            t2 = stream2.tile([128, 3200], BF16, tag="t2")
            # init the never-DMA'd rows so the full-width reduce below reads
            # defined data (their pool output rows are garbage, never read)
            nc.gpsimd.memset(t2[32:64, :], 0.0)
            nc.gpsimd.memset(t2[96:128, :], 0.0)
            nc.scalar.dma_start(t2[0:34, :], xp2[g, 0:34, :])
            nc.scalar.dma_start(t2[64:98, :], xp2[g, 34:68, :])
            if g in (0, 1, 3, 4) and w4_fetched < 5:
                w4_fetch(w4_fetched)
                w4_fetched += 1

            # pool: one reduce per chunk (xp1) + one per group (xp2 pair)
            t1v = t1[:].rearrange("p (ch col s) -> p ch col s", ch=2, s=8)
            po2 = pools.tile([128, 400], BF16, tag="po2")
            nc.vector.tensor_reduce(
                po2[:], t2[:].rearrange("p (col s) -> p col s", s=8),
                axis=mybir.AxisListType.X, op=ALU.max)
            po1a = pools.tile([128, 400], BF16, tag="po1a")
            nc.vector.tensor_reduce(po1a[:], t1v[:, 0],
                                    axis=mybir.AxisListType.X, op=ALU.max)
            po1b = pools.tile([128, 400], BF16, tag="po1b")
            nc.vector.tensor_reduce(po1b[:], t1v[:, 1],
                                    axis=mybir.AxisListType.X, op=ALU.max)

            # conv1: two chunks, K = 128 + 34; stationary columns duplicated
            # (M=128 = two copies of the 64 och) so the PE itself produces
            # both h1 halves lane-aligned.
            p1a = ps1.tile([128, 400], F32, tag="p1a")
            nc.tensor.matmul(p1a[:], w1a_t[:], po1a[:], start=True, stop=False)
            nc.tensor.matmul(p1a[:], w1b_t[:], po2[0:34, :], start=False, stop=True)
            p1b = ps1.tile([128, 400], F32, tag="p1b")
            nc.tensor.matmul(p1b[:], w1a_t[:], po1b[:], start=True, stop=False)
            nc.tensor.matmul(p1b[:], w1bb_t[64:98, :], po2[64:98, :],
                             start=False, stop=True)

            # h1 writes (relu+bias), double-written with the -4 col shift
            for k, p1 in ((0, p1a), (1, p1b)):
                c0 = (2 * g + k) * PC
                nc.scalar.activation(h1[0:64, c0 + 4:c0 + 4 + PC], p1[0:64, :],
                                     AF.Relu, bias=b1_t[0:64, 0:1])
                nc.scalar.activation(h1[64:128, c0:c0 + PC], p1[64:128, :],
                                     AF.Relu, bias=b1_t[64:128, 0:1])

            # conv2 + conv4 batches
            for (bg, s0, s1, q0, q1) in BATCHES:
                if bg != g:
                    continue
                ns = s1 - s0
                p2 = ps2.tile([128, ns * 40], F32, tag="p2")
                li = 0
                for lt in range(2):
                    for lx in range(2):
                        for ly in range(2):
                            rhs = h1v[:, s0:s1, :, lt, lx, ly, 0, :]
                            nc.tensor.matmul(
                                p2[:], w2p_t[:, li * 128:(li + 1) * 128], rhs,
                                start=(li == 0), stop=(li == 7))
                            li += 1
                nc.scalar.activation(h2[:, s0 * 40:s1 * 40], p2[:],
                                     AF.Relu, bias=b2_t[:, 0:1])
                for q in range(q0, q1):
                    nc.tensor.matmul(
                        p4[:], h2[:, 4 * q:4 * q + 4], w4v[:, q],
                        start=(q == 0), stop=(q == 99))

        # ---- AllReduce conv4 partials: [4, 256] f32 = 4KB ----
        sb4 = big.tile([4, 256], F32, tag="sb4")
        nc.scalar.activation(sb4[:], p4[:], AF.Copy)
        ar_in = dram.tile([4, 256], F32, tag="ari")
        ar_out = dram.tile([4, 256], F32, tag="aro")
        sync.dma_start(ar_in[:], sb4[:])
        nc.gpsimd.collective_compute(
            "AllReduce", ALU.add,
            replica_groups=[list(range(8))],
            ins=[ar_in[:].opt()], outs=[ar_out[:].opt()])
        h256 = big.tile([4, 256], F32, tag="h256")
        sync.dma_start(h256[:], ar_out[:])

        # transpose [4, 256] -> two [128, 4] halves via matmul with eye4
        h256r = big.tile([128, 8], F32, tag="h256r")
        for hh in range(2):
            tp = pst.tile([128, 4], F32, tag="tp")
            nc.tensor.matmul(tp[:], h256[:, 128 * hh:128 * hh + 128], eye_t[:],
                             start=True, stop=True)
            nc.scalar.activation(h256r[:, 4 * hh:4 * hh + 4], tp[:],
                                 AF.Relu, bias=b4_t[:, hh:hh + 1])

        # ---- MLP head (redundant on every core) ----
        pl = pst.tile([128, 16], F32, tag="pl")
        for s in range(4):
            msl = slice(128 * s, 128 * (s + 1))
            pls = pl[:, 4 * s:4 * s + 4]
            nc.tensor.matmul(pls, wl0_t[:, msl], jnt_t[:], start=True, stop=False)
            nc.tensor.matmul(pls, wl1_t[:, msl], h256r[:, 0:4], start=False, stop=False)
            nc.tensor.matmul(pls, wl2_t[:, msl], h256r[:, 4:8], start=False, stop=True)
        hl = big.tile([128, 16], F32, tag="hl")
        for s in range(4):
            nc.scalar.activation(hl[:, 4 * s:4 * s + 4], pl[:, 4 * s:4 * s + 4],
                                 AF.Relu, bias=bl_t[:, s:s + 1])
        wo_v = wo_t[:].rearrange("p (s n) -> p s n", s=4)
        po = pst.tile([128, 16], F32, tag="pl")  # reuse pl's bank (WAR via ring)
        for s in range(4):
            nc.tensor.matmul(po[0:6, 0:4], wo_v[:, s, :], hl[:, 4 * s:4 * s + 4],
                             start=(s == 0), stop=(s == 3))
        ot = big.tile([6, 4], F32, tag="ot")
        nc.scalar.activation(ot[:], po[0:6, 0:4], AF.Tanh, bias=bo_t[:, 0:1])
        sync.dma_start(out, ot[:])


def _prep(x, jnt_pos, jnt_goal, w1, b1, g1, be1, m1, v1, w2, b2, g2, be2, m2, v2,
          w4, b4, gn, ben, mn, vn, wl, bl, gl, bel, ml, vl, wo, bo):
    """Host-side shard + layout prep (layout/precision only). 8 in_maps."""
    f = np.float32

    def fold(w, b, g, be, m, v):
        s = (g / np.sqrt(v + EPS)).astype(f)
        return (w * s.reshape((-1,) + (1,) * (w.ndim - 1))).astype(f), \
               ((b - m) * s + be).astype(f)

    w1f, b1f = fold(w1, b1, g1, be1, m1, v1)
    w2f, b2f = fold(w2, b2, g2, be2, m2, v2)
    w4f, b4f = fold(w4, b4, gn, ben, mn, vn)
    wlf, blf = fold(wl, bl, gl, bel, ml, vl)

    # conv1 lhsT [162, 128], K order (c, kt, kx, ky, kz); och duplicated so
    # one matmul writes both h1 partition halves
    w1k = np.ascontiguousarray(
        w1f.transpose(1, 2, 3, 4, 5, 0).reshape(162, 64)).astype(BF)
    w1k = np.ascontiguousarray(np.concatenate([w1k, w1k], axis=1))
    # conv2 lhsT tap-pairs: j = (kt, kx, ky); rows 0..63 kz=0, 64..127 kz=1
    w2j = np.empty((8, 128, 128), dtype=f)
    for j in range(8):
        kt, kx, ky = (j >> 2) & 1, (j >> 1) & 1, j & 1
        w2j[j, 0:64] = w2f[:, :, kt, kx, ky, 0].T
        w2j[j, 64:128] = w2f[:, :, kt, kx, ky, 1].T
    w2p = np.ascontiguousarray(
        w2j.transpose(1, 0, 2).reshape(128, 8 * 128)).astype(BF)
    # MLP
    wlT = np.ascontiguousarray(wlf.T)          # [268, 512]
    wl0 = np.ascontiguousarray(wlT[0:12])
    wl1 = np.ascontiguousarray(wlT[12:140])
    wl2 = np.ascontiguousarray(wlT[140:268])
    bl4 = np.ascontiguousarray(blf.reshape(4, 128).T)     # [128, 4]
    wo4 = np.ascontiguousarray(
        wo.T.reshape(4, 128, 6).transpose(1, 0, 2).reshape(128, 24).astype(f))
    bo1 = np.ascontiguousarray(bo.reshape(6, 1).astype(f))
    b4r = np.ascontiguousarray(b4f.reshape(2, 128).T)     # [128, 2]
    jntc = np.ascontiguousarray(
        np.concatenate([jnt_pos, jnt_goal], axis=1).T.astype(f))  # [12, 4]

    shared = dict(
        w1a=np.ascontiguousarray(w1k[:128]),
        w1b=np.ascontiguousarray(w1k[128:]),
        b1=np.ascontiguousarray(np.concatenate([b1f, b1f]).reshape(128, 1)),
        w2p=w2p, b2=b2f.reshape(128, 1),
        b4=b4r, eye4=np.eye(4, dtype=f),
        wl0=wl0, wl1=wl1, wl2=wl2, bl=bl4, wo=wo4, bo=bo1, jnt=jntc,
    )

    xb = x.astype(BF)  # one contiguous cast, then per-core strided gathers
    in_maps = []
    for i in range(8):
        xs = xb[:, :, :, :, :, 12 * i:12 * (i + 1)]
        # axes: b0 c1 pt2 kt3 qx4 lx5 kx6 sx7 qy8 ly9 ky10 sy11 lz12 kz13 sz14
        xr = xs.reshape(4, 2, 2, 3, 10, 2, 3, 2, 10, 2, 3, 2, 2, 3, 2)
        # -> [K(c,kt,kx,ky,kz)=162, col(qx,qy,pt,lx,ly,lz,b)=6400, slab(sx,sy,sz)=8]
        xp = xr.transpose(1, 3, 6, 10, 13, 4, 8, 2, 5, 9, 12, 0, 7, 11, 14)
        xp = np.ascontiguousarray(xp).reshape(162, 6400, 8)
        xp1 = np.ascontiguousarray(xp[:128]).reshape(128, 8, 6400)
        # xp2: [8 groups, 68, 3200]; rows 0:34 chunk 2g, 34:68 chunk 2g+1
        xp2 = np.ascontiguousarray(
            xp[128:].reshape(34, 8, 2, 3200).transpose(1, 2, 0, 3)
        ).reshape(8, 68, 3200)
        # conv4 weights: core's qz=i slice -> [128c, (qx,qy)=100, 256o]
        w4q = np.ascontiguousarray(
            w4f[:, :, 0, :, :, i].transpose(1, 2, 3, 0).reshape(128, 100 * 256)
        ).astype(BF)
        m = dict(shared)
        m["xp1"] = xp1
        m["xp2"] = xp2
        m["w4"] = w4q
        in_maps.append(m)
    return in_maps


def kernel(**inputs):
    global LAST_EXEC_NS
    if "nc" not in _CACHE:
        _CACHE["nc"] = _build()
    nc = _CACHE["nc"]
    in_maps = _prep(**inputs)
    tr = bool(_CACHE.get("trace"))
    kw = {}
    if tr:
        import shutil
        shutil.rmtree("/tmp/ktrace", ignore_errors=True)
        import os as _os
        _os.makedirs("/tmp/ktrace", exist_ok=True)
        kw["tmpdir"] = "/tmp/ktrace"
    res = run_bass_kernel_spmd(nc, in_maps, core_ids=list(range(8)),
                               trace=tr, **kw)
    LAST_EXEC_NS = res.exec_time_ns
    out = res.results[0]["out"]           # [6, 4]
    return np.ascontiguousarray(out.T)    # [4, 6]


# revision 31
# speedup vs baseline: 1.0110x; 1.0110x over previous
"""Trainium2 Bass kernel for nn_Actor (4D strided Minkowski-style conv net + MLP head).

v3: instruction-count overhaul of the z-block-sharded scheme (Z=96 -> 8 blocks
of 12; core i owns conv2-z slice i, conv4 partials AllReduced, 4KB f32).

Key structural choices (driven by trace analysis of v2 @207us):
- Pool = ONE nc.vector.tensor_reduce per chunk (slab-last host layout), not a
  3-op DVE tree. xp2 (K rows 128..161) packs 2 chunks at partition bases 0 and
  64 of one [128, 3200] tile so ONE reduce covers both chunks (DVE cost is
  free-dim-bound; rows 34..63 / 98..127 are garbage and never read). The
  chunk-b conv1 matmul runs at partition base 64 (legal for K<=64) against a
  second copy of w1b parked at partitions 64..97.
- conv1: PC=400 cols/chunk, 16 chunks, 2 chunks per 1.64MB DMA.
- conv2: tap-pairs (kz 0/1) packed into K=128 stationaries. h1 is written
  twice: rows 0..63 at col offset +4, rows 64..127 at offset 0, so one
  rectangular rhs view reads tap kz=0 from the lower half and kz=1 from the
  upper half. 8 matmuls per stripe-batch, 3 batches (stripes 0-4, 5-7, 8-9).
- conv4 flipped: stationary = h2[:, 4q:4q+4] (K=128 ch, M=4 batch), moving =
  w4 slice [128, 256 och] -> out [4, 256] PSUM accumulated over q. 100
  LDW+MM pairs with 256-col moving operands (vs 200 pairs of 4-col MMs).
- MLP head identical to v2; h256 [4,256] transposed to 2x[128,4] via
  matmul-with-eye4 before the BN bias/relu.

All heavy tensors bf16 (fp32 PSUM accumulation), BN folded into conv/linear
weights host-side. Host prep does layout/precision only.
"""

import sys

sys.path.insert(0, "/opt/trn_rl_repo")

from contextlib import ExitStack

import ml_dtypes
import numpy as np

import concourse.bass as bass
import concourse.tile as tile
from concourse import bacc, mybir
from concourse.bass_utils import run_bass_kernel_spmd

EPS = 1e-5
F32 = mybir.dt.float32
BF16 = mybir.dt.bfloat16
AF = mybir.ActivationFunctionType
ALU = mybir.AluOpType
BF = ml_dtypes.bfloat16

PC = 400          # patch columns per chunk
NCH = 16          # chunks
NG = 8            # DMA/pool groups (2 chunks each)
# conv2/conv4 batches: (after group, stripe range, q range)
BATCHES = [(3, 0, 5, 0, 50), (6, 5, 8, 50, 80), (7, 8, 10, 80, 100)]

LAST_EXEC_NS = None
LAST_MEAN_NS = None
_CACHE = {}


def _build():
    nc = bacc.Bacc(
        "TRN2",
        target_bir_lowering=False,
        debug=False,
        enable_asserts=False,
        num_devices=8,
    )
    d = {}

    def din(name, shape, dt=BF16):
        d[name] = nc.dram_tensor(name, list(shape), dt, kind="ExternalInput").ap()
        return d[name]

    xp1 = din("xp1", (128, NG, 6400))
    xp2 = din("xp2", (NG // 2, 68, 6400))
    w1a = din("w1a", (128, 128))
    w1b = din("w1b", (34, 128))
    b1 = din("b1", (128, 1), F32)
    w2p = din("w2p", (128, 8 * 128))
    b2 = din("b2", (128, 1), F32)
    w4 = din("w4", (128, 100 * 256))
    b4 = din("b4", (128, 2), F32)
    eye4 = din("eye4", (4, 4), F32)
    wl0 = din("wl0", (12, 512), F32)
    wl1 = din("wl1", (128, 512), F32)
    wl2 = din("wl2", (128, 512), F32)
    bl = din("bl", (128, 4), F32)
    wo = din("wo", (128, 24), F32)
    bo = din("bo", (6, 1), F32)
    jnt = din("jnt", (12, 4), F32)
    out = nc.dram_tensor("out", [6, 4], F32, kind="ExternalOutput").ap()

    with TileKernel(nc) as tk:
        tk.run(xp1, xp2, w1a, w1b, b1, w2p, b2, w4, b4, eye4,
               wl0, wl1, wl2, bl, wo, bo, jnt, out)
    nc.compile()
    return nc


class TileKernel:
    def __init__(self, nc):
        self.nc = nc
        self.tc = tile.TileContext(nc)
        self.ctx = ExitStack()

    def __enter__(self):
        self.tc.__enter__()
        return self

    def __exit__(self, *a):
        self.ctx.close()
        return self.tc.__exit__(*a)

    def run(self, xp1, xp2, w1a, w1b, b1, w2p, b2, w4, b4, eye4,
            wl0, wl1, wl2, bl, wo, bo, jnt, out):
        nc, tc, ctx = self.nc, self.tc, self.ctx
        sync = nc.sync

        const = ctx.enter_context(tc.tile_pool(name="const", bufs=1))
        stream1 = ctx.enter_context(tc.tile_pool(name="stream1", bufs=3))
        stream2 = ctx.enter_context(tc.tile_pool(name="stream2", bufs=2))
        scratch = ctx.enter_context(tc.tile_pool(name="scratch", bufs=2))
        pools = ctx.enter_context(tc.tile_pool(name="pools", bufs=3))
        big = ctx.enter_context(tc.tile_pool(name="big", bufs=1))
        self.scratch = scratch
        ps1 = ctx.enter_context(tc.tile_pool(name="ps1", bufs=2, space="PSUM"))
        ps2 = ctx.enter_context(tc.tile_pool(name="ps2", bufs=1, space="PSUM"))
        ps4 = ctx.enter_context(tc.tile_pool(name="ps4", bufs=1, space="PSUM"))
        pst = ctx.enter_context(tc.tile_pool(name="pst", bufs=1, space="PSUM"))
        dram = ctx.enter_context(tc.tile_pool(name="dram", bufs=1, space="DRAM"))

        # semaphores for the hand-rolled all-reduce exchange; cleared first
        # thing (no peer can deliver this early: sends happen ~100us in,
        # launch skew is well below that)
        rsem = nc.alloc_semaphore("ar_recv")
        lsem = nc.alloc_semaphore("ar_local")
        nc.gpsimd.sem_clear(rsem)
        nc.gpsimd.sem_clear(lsem)

        # ---- constants into SBUF ----
        w1a_t = const.tile([128, 128], BF16); sync.dma_start(w1a_t[:], w1a)
        w1b_t = const.tile([34, 128], BF16); sync.dma_start(w1b_t[:], w1b)
        w1bb_t = const.tile([98, 128], BF16)
        sync.dma_start(w1bb_t[64:98, :], w1b)
        b1_t = const.tile([128, 1], F32); sync.dma_start(b1_t[:], b1)
        w2p_t = const.tile([128, 8 * 128], BF16); sync.dma_start(w2p_t[:], w2p)
        b2_t = const.tile([128, 1], F32); sync.dma_start(b2_t[:], b2)
        b4_t = const.tile([128, 2], F32); sync.dma_start(b4_t[:], b4)
        eye_t = const.tile([4, 4], F32); sync.dma_start(eye_t[:], eye4)
        wl0_t = const.tile([12, 512], F32); sync.dma_start(wl0_t[:], wl0)
        wl1_t = const.tile([128, 512], F32); sync.dma_start(wl1_t[:], wl1)
        wl2_t = const.tile([128, 512], F32); sync.dma_start(wl2_t[:], wl2)
        bl_t = const.tile([128, 4], F32); sync.dma_start(bl_t[:], bl)
        wo_t = const.tile([128, 24], F32); sync.dma_start(wo_t[:], wo)
        bo_t = const.tile([6, 1], F32); sync.dma_start(bo_t[:], bo)
        jnt_t = const.tile([12, 4], F32); sync.dma_start(jnt_t[:], jnt)

        # w4 prefetched in 5 stripes of [128, 5120] on the scalar ring
        w4_t = const.tile([128, 100 * 256], BF16)
        SW = 20 * 256

        def w4_fetch(s):
            nc.scalar.dma_start(w4_t[:, s * SW:(s + 1) * SW],
                                w4[:, s * SW:(s + 1) * SW])

        w4_fetch(0)
        w4v = w4_t[:].rearrange("p (q o) -> p q o", q=100)

        # h1: [128, 6404] bf16. rows 0..63 hold data at tile col = data+4,
        # rows 64..127 hold the same data at tile col = data. One rectangular
        # conv2 rhs view then pairs tap kz=0 (lower) with kz=1 (upper).
        h1 = big.tile([128, 6404], BF16)
        # data-col view (tile cols 4..6403 on the lower half's coordinates)
        h1v = h1[:, 4:6404].rearrange(
            "p (qx qy pt lx ly lz b) -> p qx qy pt lx ly lz b",
            qx=10, qy=10, pt=2, lx=2, ly=2, lz=2)
        h2 = big.tile([128, 400], BF16)       # col = q*4 + b
        p4 = ps4.tile([4, 256], F32)          # conv4 acc: [batch, och]

        w4_fetched = 1

        # ---- streaming phase ----
        # super-group h = groups (2h, 2h+1) = chunks 4h..4h+3; xp2 packs the
        # two groups' K-rows at partition bases 0 and 64 of one [128, 6400]
        # tile so one 3-op pool tree covers 4 chunks.
        po2 = None
        for g in range(NG):
            h = g // 2
            if g % 2 == 0:
                t2 = stream2.tile([128, 6400], BF16, tag="t2")
                # init the never-DMA'd rows so the full-width tree below
                # reads defined data (their pool rows are garbage, unread)
                nc.gpsimd.memset(t2[32:64, :], 0.0)
                nc.gpsimd.memset(t2[96:128, :], 0.0)
                nc.scalar.dma_start(t2[0:34, :], xp2[h, 0:34, :])
                nc.scalar.dma_start(t2[64:98, :], xp2[h, 34:68, :])
                po2 = self._pooltree(pools, t2, 800, "b")
            t1 = stream1.tile([128, 6400], BF16, tag="t1")
            sync.dma_start(t1[:], xp1[:, g, :])
            if g in (0, 1, 3, 4) and w4_fetched < 5:
                w4_fetch(w4_fetched)
                w4_fetched += 1
            po1 = self._pooltree(pools, t1, 800, "a")

            # conv1: two chunks, K = 128 + 34; stationary columns duplicated
            # (M=128 = two copies of the 64 och) so the PE itself produces
            # both h1 halves lane-aligned.
            blk = 64 * (g % 2)
            w1bx = w1bb_t[64:98, :] if blk else w1b_t[:]
            for k in range(2):
                p1 = ps1.tile([128, 400], F32, tag=f"p1{k}")
                nc.tensor.matmul(p1[:], w1a_t[:], po1[:, 400 * k:400 * k + 400],
                                 start=True, stop=False)
                nc.tensor.matmul(p1[:], w1bx,
                                 po2[blk:blk + 34, 400 * k:400 * k + 400],
                                 start=False, stop=True)
                c0 = (2 * g + k) * PC
                nc.scalar.activation(h1[0:64, c0 + 4:c0 + 4 + PC], p1[0:64, :],
                                     AF.Relu, bias=b1_t[0:64, 0:1])
                nc.scalar.activation(h1[64:128, c0:c0 + PC], p1[64:128, :],
                                     AF.Relu, bias=b1_t[64:128, 0:1])

            # conv2 + conv4 batches
            for (bg, s0, s1, q0, q1) in BATCHES:
                if bg != g:
                    continue
                ns = s1 - s0
                p2 = ps2.tile([128, ns * 40], F32, tag="p2")
                li = 0
                for lt in range(2):
                    for lx in range(2):
                        for ly in range(2):
                            rhs = h1v[:, s0:s1, :, lt, lx, ly, 0, :]
                            nc.tensor.matmul(
                                p2[:], w2p_t[:, li * 128:(li + 1) * 128], rhs,
                                start=(li == 0), stop=(li == 7))
                            li += 1
                nc.scalar.activation(h2[:, s0 * 40:s1 * 40], p2[:],
                                     AF.Relu, bias=b2_t[:, 0:1])
                for q in range(q0, q1):
                    nc.tensor.matmul(
                        p4[:], h2[:, 4 * q:4 * q + 4], w4v[:, q],
                        start=(q == 0), stop=(q == 99))

        # ---- hand-rolled all-reduce of conv4 partials ----
        # transpose own partial [4, 256] -> [128, 8] BEFORE the exchange
        # (transpose commutes with the sum), then every core sends its
        # [128, 8] f32 partial to all 8 cores via single-dest relative
        # remote DMAs: call D targets XOR-peer (0, D) and writes slot D, so
        # receiver r's slot D holds sender r^D -- all 8 partials land, in a
        # per-core order, which the sum doesn't care about.
        partial_r = big.tile([128, 8], F32, tag="partial_r")
        recv = big.tile([128, 64], F32, tag="recv")
        for dd in range(8):
            rd = [None] * 8
            rd[dd] = (0, dd)
            nc.gpsimd.remote_dma_broadcast(
                recv[:, 8 * dd:8 * dd + 8], partial_r[:], rsem, lsem, rdests=rd)
        sb4 = big.tile([4, 256], F32, tag="sb4")
        nc.scalar.activation(sb4[:], p4[:], AF.Copy)
        for hh in range(2):
            tp = pst.tile([128, 4], F32, tag="tp")
            nc.tensor.matmul(tp[:], sb4[:, 128 * hh:128 * hh + 128], eye_t[:],
                             start=True, stop=True)
            nc.scalar.activation(partial_r[:, 4 * hh:4 * hh + 4], tp[:], AF.Copy)
        nc.gpsimd.trigger_dma(count=None)

        # wait for all 8 deliveries (2 sem incs each), then tree-sum slots
        nc.vector.wait_ge(rsem, 16)
        rv = recv[:].rearrange("p (s f) -> p s f", s=8)
        s4 = big.tile([128, 32], F32, tag="s4")
        s4v = s4[:].rearrange("p (s f) -> p s f", s=4)
        nc.vector.tensor_tensor(s4v, rv[:, 0:4, :], rv[:, 4:8, :], op=ALU.add)
        s2 = big.tile([128, 16], F32, tag="s2")
        s2v = s2[:].rearrange("p (s f) -> p s f", s=2)
        nc.vector.tensor_tensor(s2v, s4v[:, 0:2, :], s4v[:, 2:4, :], op=ALU.add)
        h256s = big.tile([128, 8], F32, tag="h256s")
        nc.vector.tensor_tensor(h256s[:], s2v[:, 0, :], s2v[:, 1, :], op=ALU.add)

        h256r = big.tile([128, 8], F32, tag="h256r")
        for hh in range(2):
            nc.scalar.activation(h256r[:, 4 * hh:4 * hh + 4],
                                 h256s[:, 4 * hh:4 * hh + 4],
                                 AF.Relu, bias=b4_t[:, hh:hh + 1])

        # ---- MLP head (redundant on every core) ----
        pl = pst.tile([128, 16], F32, tag="pl")
        for s in range(4):
            msl = slice(128 * s, 128 * (s + 1))
            pls = pl[:, 4 * s:4 * s + 4]
            nc.tensor.matmul(pls, wl0_t[:, msl], jnt_t[:], start=True, stop=False)
            nc.tensor.matmul(pls, wl1_t[:, msl], h256r[:, 0:4], start=False, stop=False)
            nc.tensor.matmul(pls, wl2_t[:, msl], h256r[:, 4:8], start=False, stop=True)
        hl = big.tile([128, 16], F32, tag="hl")
        for s in range(4):
            nc.scalar.activation(hl[:, 4 * s:4 * s + 4], pl[:, 4 * s:4 * s + 4],
                                 AF.Relu, bias=bl_t[:, s:s + 1])
        wo_v = wo_t[:].rearrange("p (s n) -> p s n", s=4)
        po = pst.tile([128, 16], F32, tag="pl")  # reuse pl's bank (WAR via ring)
        for s in range(4):
            nc.tensor.matmul(po[0:6, 0:4], wo_v[:, s, :], hl[:, 4 * s:4 * s + 4],
                             start=(s == 0), stop=(s == 3))
        ot = big.tile([6, 4], F32, tag="ot")
        nc.scalar.activation(ot[:], po[0:6, 0:4], AF.Tanh, bias=bo_t[:, 0:1])
        sync.dma_start(out, ot[:])

    def _pooltree(self, outpool, t, f, tag):
        """max over 8 slab blocks: t [128, 8*f] slab-major -> [128, f]."""
        nc = self.nc
        v8 = t[:].rearrange("p (x c f) -> p x c f", x=4, c=2)
        m4 = self.scratch.tile([128, 4 * f], BF16, tag=tag + "4")
        m4v = m4[:].rearrange("p (x f) -> p x f", x=4)
        nc.vector.tensor_tensor(m4v, v8[:, :, 0, :], v8[:, :, 1, :], op=ALU.max)
        v4 = m4[:].rearrange("p (x c f) -> p x c f", x=2, c=2)
        m2 = self.scratch.tile([128, 2 * f], BF16, tag=tag + "2")
        m2v = m2[:].rearrange("p (x f) -> p x f", x=2)
        nc.vector.tensor_tensor(m2v, v4[:, :, 0, :], v4[:, :, 1, :], op=ALU.max)
        v2 = m2[:].rearrange("p (c f) -> p c f", c=2)
        m1 = outpool.tile([128, f], BF16, tag=tag + "1")
        nc.vector.tensor_tensor(m1[:], v2[:, 0, :], v2[:, 1, :], op=ALU.max)
        return m1


def _prep(x, jnt_pos, jnt_goal, w1, b1, g1, be1, m1, v1, w2, b2, g2, be2, m2, v2,
          w4, b4, gn, ben, mn, vn, wl, bl, gl, bel, ml, vl, wo, bo):
    """Host-side shard + layout prep (layout/precision only). 8 in_maps."""
    f = np.float32

    def fold(w, b, g, be, m, v):
        s = (g / np.sqrt(v + EPS)).astype(f)
        return (w * s.reshape((-1,) + (1,) * (w.ndim - 1))).astype(f), \
               ((b - m) * s + be).astype(f)

    w1f, b1f = fold(w1, b1, g1, be1, m1, v1)
    w2f, b2f = fold(w2, b2, g2, be2, m2, v2)
    w4f, b4f = fold(w4, b4, gn, ben, mn, vn)
    wlf, blf = fold(wl, bl, gl, bel, ml, vl)

    # conv1 lhsT [162, 128], K order (c, kt, kx, ky, kz); och duplicated so
    # one matmul writes both h1 partition halves
    w1k = np.ascontiguousarray(
        w1f.transpose(1, 2, 3, 4, 5, 0).reshape(162, 64)).astype(BF)
    w1k = np.ascontiguousarray(np.concatenate([w1k, w1k], axis=1))
    # conv2 lhsT tap-pairs: j = (kt, kx, ky); rows 0..63 kz=0, 64..127 kz=1
    w2j = np.empty((8, 128, 128), dtype=f)
    for j in range(8):
        kt, kx, ky = (j >> 2) & 1, (j >> 1) & 1, j & 1
        w2j[j, 0:64] = w2f[:, :, kt, kx, ky, 0].T
        w2j[j, 64:128] = w2f[:, :, kt, kx, ky, 1].T
    w2p = np.ascontiguousarray(
        w2j.transpose(1, 0, 2).reshape(128, 8 * 128)).astype(BF)
    # MLP
    wlT = np.ascontiguousarray(wlf.T)          # [268, 512]
    wl0 = np.ascontiguousarray(wlT[0:12])
    wl1 = np.ascontiguousarray(wlT[12:140])
    wl2 = np.ascontiguousarray(wlT[140:268])
    bl4 = np.ascontiguousarray(blf.reshape(4, 128).T)     # [128, 4]
    wo4 = np.ascontiguousarray(
        wo.T.reshape(4, 128, 6).transpose(1, 0, 2).reshape(128, 24).astype(f))
    bo1 = np.ascontiguousarray(bo.reshape(6, 1).astype(f))
    b4r = np.ascontiguousarray(b4f.reshape(2, 128).T)     # [128, 2]
    jntc = np.ascontiguousarray(
        np.concatenate([jnt_pos, jnt_goal], axis=1).T.astype(f))  # [12, 4]

    shared = dict(
        w1a=np.ascontiguousarray(w1k[:128]),
        w1b=np.ascontiguousarray(w1k[128:]),
        b1=np.ascontiguousarray(np.concatenate([b1f, b1f]).reshape(128, 1)),
        w2p=w2p, b2=b2f.reshape(128, 1),
        b4=b4r, eye4=np.eye(4, dtype=f),
        wl0=wl0, wl1=wl1, wl2=wl2, bl=bl4, wo=wo4, bo=bo1, jnt=jntc,
    )

    xb = x.astype(BF)  # one contiguous cast, then per-core strided gathers
    in_maps = []
    for i in range(8):
        xs = xb[:, :, :, :, :, 12 * i:12 * (i + 1)]
        # axes: b0 c1 pt2 kt3 qx4 lx5 kx6 sx7 qy8 ly9 ky10 sy11 lz12 kz13 sz14
        xr = xs.reshape(4, 2, 2, 3, 10, 2, 3, 2, 10, 2, 3, 2, 2, 3, 2)
        # -> [K(c,kt,kx,ky,kz)=162, slab(sx,sy,sz)=8, col(qx,qy,pt,lx,ly,lz,b)=6400]
        xp = xr.transpose(1, 3, 6, 10, 13, 7, 11, 14, 4, 8, 2, 5, 9, 12, 0)
        xp = np.ascontiguousarray(xp).reshape(162, 8, 8, 800)
        # slab-major WITHIN each 800-col group: [162, group, slab, 800]
        xpg = np.ascontiguousarray(xp.transpose(0, 2, 1, 3))
        xp1 = np.ascontiguousarray(xpg[:128]).reshape(128, 8, 6400)
        # xp2: [4 super-groups, 68, 6400]; rows 0:34 group 2h, 34:68 group 2h+1
        xp2 = np.ascontiguousarray(
            xpg[128:].reshape(34, 4, 2, 6400).transpose(1, 2, 0, 3)
        ).reshape(4, 68, 6400)
        # conv4 weights: core's qz=i slice -> [128c, (qx,qy)=100, 256o]
        w4q = np.ascontiguousarray(
            w4f[:, :, 0, :, :, i].transpose(1, 2, 3, 0).reshape(128, 100 * 256)
        ).astype(BF)
        m = dict(shared)
        m["xp1"] = xp1
        m["xp2"] = xp2
        m["w4"] = w4q
        in_maps.append(m)
    return in_maps


def kernel(**inputs):
    global LAST_EXEC_NS
    if "nc" not in _CACHE:
        _CACHE["nc"] = _build()
    nc = _CACHE["nc"]
    in_maps = _prep(**inputs)
    tr = bool(_CACHE.get("trace"))
    kw = {}
    if tr:
        import shutil
        shutil.rmtree("/tmp/ktrace", ignore_errors=True)
        import os as _os
        _os.makedirs("/tmp/ktrace", exist_ok=True)
        kw["tmpdir"] = "/tmp/ktrace"
    res = run_bass_kernel_spmd(nc, in_maps, core_ids=list(range(8)),
                               trace=tr, **kw)
    LAST_EXEC_NS = res.exec_time_ns
    global LAST_MEAN_NS
    LAST_MEAN_NS = res.mean_exec_time_ns
    out = res.results[0]["out"]           # [6, 4]
    return np.ascontiguousarray(out.T)    # [4, 6]


# revision 32
# speedup vs baseline: 1.0665x; 1.0549x over previous
"""Trainium2 Bass kernel for nn_Actor (4D strided Minkowski-style conv net + MLP head).

Sharding v2: Z-block sharding. Z=96 splits into exactly 8 conv2-aligned blocks
of 12, so core i processes x[..., 12i:12(i+1)] for ALL 4 batches and locally
produces h2 for its own conv4 q-slice (qz=i). That makes conv4 q-sharded with
zero cross-core exchange before it; a single 4KB AllReduce of the conv4
partials replaces both AllGathers of the batch-sharded scheme.

conv4 streams w4 (the 105MB tensor, och-paged per q) through the PE as the
stationary operand in 200 [128,128] loads while h2 columns move; w4 is
prefetched into SBUF during the x-streaming phase so the tail is compute-only.

All heavy tensors are cast to bf16 host-side (halves HBM traffic; fp32 PSUM
accumulation keeps rel err ~9e-3, tolerance 2e-2). All convs have
kernel == stride, so each conv is a patch matmul. Host-side numpy only
rearranges layout / folds BN into conv weights (pure weight preprocessing).

Per-chunk pipeline: each 320-patch chunk is one merged DMA per input slab
group; pool tree runs on DVE (128-row part) and GpSimd (34-row part); after
every qx-stripe (2 chunks) conv2 + conv4-accumulate run so only AllReduce+MLP
remain after the stream.
"""

import sys

sys.path.insert(0, "/opt/trn_rl_repo")

from contextlib import ExitStack

import ml_dtypes
import numpy as np

import concourse.bass as bass
import concourse.tile as tile
from concourse import bacc, mybir
from concourse.bass_utils import run_bass_kernel_spmd

EPS = 1e-5
F32 = mybir.dt.float32
BF16 = mybir.dt.bfloat16
AF = mybir.ActivationFunctionType
BF = ml_dtypes.bfloat16

PC = 320           # patch columns per stream chunk
NCH = 6400 // PC   # 20 chunks; 2 chunks == 1 qx stripe
NQX = 10           # qx stripes; stripe s covers q = 10s..10s+9

LAST_EXEC_NS = None
_CACHE = {}


def _build():
    nc = bacc.Bacc(
        "TRN2",
        target_bir_lowering=False,
        debug=False,
        enable_asserts=False,
        num_devices=8,
    )
    d = {}

    def din(name, shape, dt=BF16):
        d[name] = nc.dram_tensor(name, list(shape), dt, kind="ExternalInput").ap()
        return d[name]

    xp1 = din("xp1", (128, NCH, 8 * PC))
    xp2 = din("xp2", (34, NCH, 8 * PC))
    w1a = din("w1a", (128, 64))
    w1b = din("w1b", (34, 64))
    b1 = din("b1", (64, 1), F32)
    w2 = din("w2", (64, 16 * 128))
    b2 = din("b2", (128, 1), F32)
    w4 = din("w4", (128, 100 * 256))
    b4 = din("b4", (128, 2), F32)
    wl0 = din("wl0", (12, 512), F32)
    wl1 = din("wl1", (128, 512), F32)
    wl2 = din("wl2", (128, 512), F32)
    bl = din("bl", (128, 4), F32)
    wo = din("wo", (128, 24), F32)
    bo = din("bo", (6, 1), F32)
    jnt = din("jnt", (12, 4), F32)
    out = nc.dram_tensor("out", [6, 4], F32, kind="ExternalOutput").ap()

    with TileKernel(nc) as tk:
        tk.run(xp1, xp2, w1a, w1b, b1, w2, b2, w4, b4,
               wl0, wl1, wl2, bl, wo, bo, jnt, out)
    nc.compile()
    return nc


class TileKernel:
    def __init__(self, nc):
        self.nc = nc
        self.tc = tile.TileContext(nc)
        self.ctx = ExitStack()

    def __enter__(self):
        self.tc.__enter__()
        return self

    def __exit__(self, *a):
        self.ctx.close()
        return self.tc.__exit__(*a)

    def run(self, xp1, xp2, w1a, w1b, b1, w2, b2, w4, b4,
            wl0, wl1, wl2, bl, wo, bo, jnt, out):
        nc, tc, ctx = self.nc, self.tc, self.ctx
        sync = nc.sync

        const = ctx.enter_context(tc.tile_pool(name="const", bufs=1))
        stream = ctx.enter_context(tc.tile_pool(name="stream", bufs=5))
        pools = ctx.enter_context(tc.tile_pool(name="pools", bufs=4))
        big = ctx.enter_context(tc.tile_pool(name="big", bufs=1))
        ps1 = ctx.enter_context(tc.tile_pool(name="ps1", bufs=3, space="PSUM"))
        ps2 = ctx.enter_context(tc.tile_pool(name="ps2", bufs=2, space="PSUM"))
        ps4 = ctx.enter_context(tc.tile_pool(name="ps4", bufs=1, space="PSUM"))
        psl = ctx.enter_context(tc.tile_pool(name="psl", bufs=1, space="PSUM"))
        pso = ctx.enter_context(tc.tile_pool(name="pso", bufs=1, space="PSUM"))
        dram = ctx.enter_context(tc.tile_pool(name="dram", bufs=1, space="DRAM"))

        # ---- constants into SBUF ----
        w1a_t = const.tile([128, 64], BF16); sync.dma_start(w1a_t[:], w1a)
        w1b_t = const.tile([34, 64], BF16); sync.dma_start(w1b_t[:], w1b)
        b1_t = const.tile([64, 1], F32); sync.dma_start(b1_t[:], b1)
        w2_t = const.tile([64, 16 * 128], BF16); sync.dma_start(w2_t[:], w2)
        b2_t = const.tile([128, 1], F32); sync.dma_start(b2_t[:], b2)
        b4_t = const.tile([128, 2], F32); sync.dma_start(b4_t[:], b4)
        wl0_t = const.tile([12, 512], F32); sync.dma_start(wl0_t[:], wl0)
        wl1_t = const.tile([128, 512], F32); sync.dma_start(wl1_t[:], wl1)
        wl2_t = const.tile([128, 512], F32); sync.dma_start(wl2_t[:], wl2)
        bl_t = const.tile([128, 4], F32); sync.dma_start(bl_t[:], bl)
        wo_t = const.tile([128, 24], F32); sync.dma_start(wo_t[:], wo)
        bo_t = const.tile([6, 1], F32); sync.dma_start(bo_t[:], bo)
        jnt_t = const.tile([12, 4], F32); sync.dma_start(jnt_t[:], jnt)

        # w4: prefetched stripe-by-stripe during the stream phase (scalar
        # HWDGE ring, so it never queues behind the x-stream triggers)
        w4_t = const.tile([128, 100 * 256], BF16)
        SW = 10 * 256  # cols per stripe

        def w4_fetch(s):
            nc.scalar.dma_start(w4_t[:, s * SW:(s + 1) * SW],
                                w4[:, s * SW:(s + 1) * SW])

        w4_fetch(0)
        w4tv = w4_t[:].rearrange("p (q o) -> p q o", q=100)

        h1 = big.tile([64, 6400], BF16)
        # h1 col = qx*640 + qy*64 + pt*32 + lx*16 + ly*8 + lz*4 + b
        h1v = h1[:].rearrange(
            "p (qx qy pt lx ly lz b) -> p qx qy pt lx ly lz b",
            qx=10, qy=10, pt=2, lx=2, ly=2, lz=2)
        h2 = big.tile([128, 400], BF16)       # col = q*4 + b
        h2v = h2[:].rearrange("p (q b) -> p q b", b=4)
        p4 = ps4.tile([128, 8], F32)          # conv4 acc: col = 4*och_half + b

        # ---- streaming phase: pool + conv1 per chunk; conv2+conv4 per stripe ----
        for c in range(NCH):
            if c % 2 == 0 and c // 2 + 1 < NQX:
                w4_fetch(c // 2 + 1)
            t1 = stream.tile([128, 8 * PC], BF16, tag="t1")
            sync.dma_start(t1[:], xp1[:, c, :])
            t2 = stream.tile([34, 8 * PC], BF16, tag="t2")
            nc.scalar.dma_start(t2[:], xp2[:, c, :])

            # t2's small DMA lands first: pool it while t1 is still in flight
            po2 = self._pooltree(pools, t2, 34, "b")
            po1 = self._pooltree(pools, t1, 128, "a")

            p1 = ps1.tile([64, PC], F32)
            nc.tensor.matmul(p1[:], w1a_t[:], po1[:], start=True, stop=False)
            nc.tensor.matmul(p1[:], w1b_t[:], po2[:], start=False, stop=True)
            nc.scalar.activation(h1[:, c * PC:(c + 1) * PC], p1[:],
                                 AF.Relu, bias=b1_t[:, 0:1])

            if c % 2 == 1:
                qx = c // 2
                # conv2 stripe: 16 shifted matmuls -> [128, 40] (qy, b)
                p2 = ps2.tile([128, 40], F32)
                li = 0
                for lt in range(2):
                    for lx in range(2):
                        for ly in range(2):
                            for lz in range(2):
                                rhs = h1v[:, qx, :, lt, lx, ly, lz, :]
                                nc.tensor.matmul(
                                    p2[:], w2_t[:, li * 128:(li + 1) * 128], rhs,
                                    start=(li == 0), stop=(li == 15))
                                li += 1
                nc.scalar.activation(h2[:, qx * 40:(qx + 1) * 40], p2[:],
                                     AF.Relu, bias=b2_t[:, 0:1])
                # conv4 stripe: accumulate q = 10*qx .. 10*qx+9
                for j in range(10):
                    q = qx * 10 + j
                    rhs4 = h2v[:, q, :]
                    for hh in range(2):
                        # NB: start=True resets the accumulate state of the
                        # whole 2KB PSUM zero region, so only the very first
                        # matmul of the group may set it.
                        nc.tensor.matmul(
                            p4[:, 4 * hh:4 * hh + 4],
                            w4tv[:, q, 128 * hh:128 * hh + 128], rhs4,
                            start=(q == 0 and hh == 0), stop=(q == 99 and hh == 1))

        # ---- AllReduce conv4 partials: [128, 8] f32 = 4KB ----
        po4 = big.tile([128, 8], F32, tag="po4")
        nc.scalar.activation(po4[:], p4[:], AF.Copy)
        ar_in = dram.tile([128, 8], F32, tag="ari")
        ar_out = dram.tile([128, 8], F32, tag="aro")
        sync.dma_start(ar_in[:], po4[:])
        nc.gpsimd.collective_compute(
            "AllReduce", mybir.AluOpType.add,
            replica_groups=[list(range(8))],
            ins=[ar_in[:].opt()], outs=[ar_out[:].opt()])
        h256 = big.tile([128, 8], F32, tag="h256")
        sync.dma_start(h256[:], ar_out[:])

        # bias + relu per och half: col = 4*half + b
        h256r = big.tile([128, 8], F32, tag="h256r")
        nc.scalar.activation(h256r[:, 0:4], h256[:, 0:4], AF.Relu, bias=b4_t[:, 0:1])
        nc.scalar.activation(h256r[:, 4:8], h256[:, 4:8], AF.Relu, bias=b4_t[:, 1:2])

        # ---- MLP head (redundant on every core) ----
        pl = psl.tile([128, 16], F32)
        for s in range(4):
            msl = slice(128 * s, 128 * (s + 1))
            pls = pl[:, 4 * s:4 * s + 4]
            nc.tensor.matmul(pls, wl0_t[:, msl], jnt_t[:], start=True, stop=False)
            nc.tensor.matmul(pls, wl1_t[:, msl], h256r[:, 0:4], start=False, stop=False)
            nc.tensor.matmul(pls, wl2_t[:, msl], h256r[:, 4:8], start=False, stop=True)
        hl = big.tile([128, 16], F32, tag="hl")
        for s in range(4):
            nc.scalar.activation(hl[:, 4 * s:4 * s + 4], pl[:, 4 * s:4 * s + 4],
                                 AF.Relu, bias=bl_t[:, s:s + 1])
        po = pso.tile([6, 4], F32)
        wo_v = wo_t[:].rearrange("p (s n) -> p s n", s=4)
        for s in range(4):
            nc.tensor.matmul(po[:], wo_v[:, s, :], hl[:, 4 * s:4 * s + 4],
                             start=(s == 0), stop=(s == 3))
        ot = big.tile([6, 4], F32, tag="ot")
        nc.scalar.activation(ot[:], po[:], AF.Tanh, bias=bo_t[:, 0:1])
        sync.dma_start(out, ot[:])

    def _pooltree(self, pool, t, p, tag):
        """max over the 8 pooled-window slabs: [p, 8, PC] -> [p, PC]."""
        nc = self.nc
        v8 = t[:].rearrange("p (x c f) -> p x c f", x=4, c=2)
        m4 = pool.tile([p, 4 * PC], BF16, tag=tag + "4")
        m4v = m4[:].rearrange("p (x f) -> p x f", x=4)
        nc.vector.tensor_tensor(m4v, v8[:, :, 0, :], v8[:, :, 1, :],
                                op=mybir.AluOpType.max)
        v4 = m4[:].rearrange("p (x c f) -> p x c f", x=2, c=2)
        m2 = pool.tile([p, 2 * PC], BF16, tag=tag + "2")
        m2v = m2[:].rearrange("p (x f) -> p x f", x=2)
        nc.vector.tensor_tensor(m2v, v4[:, :, 0, :], v4[:, :, 1, :],
                                op=mybir.AluOpType.max)
        v2 = m2[:].rearrange("p (c f) -> p c f", c=2)
        m1 = pool.tile([p, PC], BF16, tag=tag + "1")
        nc.vector.tensor_tensor(m1[:], v2[:, 0, :], v2[:, 1, :],
                                op=mybir.AluOpType.max)
        return m1



def _prep(x, jnt_pos, jnt_goal, w1, b1, g1, be1, m1, v1, w2, b2, g2, be2, m2, v2,
          w4, b4, gn, ben, mn, vn, wl, bl, gl, bel, ml, vl, wo, bo):
    """Host-side shard + layout prep (layout/precision only). 8 in_maps."""
    f = np.float32

    def fold(w, b, g, be, m, v):
        s = (g / np.sqrt(v + EPS)).astype(f)
        return (w * s.reshape((-1,) + (1,) * (w.ndim - 1))).astype(f), \
               ((b - m) * s + be).astype(f)

    w1f, b1f = fold(w1, b1, g1, be1, m1, v1)
    w2f, b2f = fold(w2, b2, g2, be2, m2, v2)
    w4f, b4f = fold(w4, b4, gn, ben, mn, vn)
    wlf, blf = fold(wl, bl, gl, bel, ml, vl)

    # conv1 lhsT [162, 64], K order (c, kt, kx, ky, kz)
    w1k = np.ascontiguousarray(
        w1f.transpose(1, 2, 3, 4, 5, 0).reshape(162, 64)).astype(BF)
    # conv2 lhsT per shift: [64, 16*128], shift order (lt, lx, ly, lz)
    w2k = np.ascontiguousarray(
        w2f.transpose(2, 3, 4, 5, 1, 0).reshape(16, 64, 128)
        .transpose(1, 0, 2).reshape(64, 16 * 128)).astype(BF)
    # MLP
    wlT = np.ascontiguousarray(wlf.T)          # [268, 512]
    wl0 = np.ascontiguousarray(wlT[0:12])
    wl1 = np.ascontiguousarray(wlT[12:140])
    wl2 = np.ascontiguousarray(wlT[140:268])
    bl4 = np.ascontiguousarray(blf.reshape(4, 128).T)     # [128, 4]
    wo4 = np.ascontiguousarray(
        wo.T.reshape(4, 128, 6).transpose(1, 0, 2).reshape(128, 24).astype(f))
    bo1 = np.ascontiguousarray(bo.reshape(6, 1).astype(f))
    b4r = np.ascontiguousarray(b4f.reshape(2, 128).T)     # [128, 2]
    jntc = np.ascontiguousarray(
        np.concatenate([jnt_pos, jnt_goal], axis=1).T.astype(f))  # [12, 4]

    shared = dict(
        w1a=np.ascontiguousarray(w1k[:128]),
        w1b=np.ascontiguousarray(w1k[128:]),
        b1=b1f.reshape(64, 1), w2=w2k, b2=b2f.reshape(128, 1),
        b4=b4r, wl0=wl0, wl1=wl1, wl2=wl2, bl=bl4, wo=wo4, bo=bo1, jnt=jntc,
    )

    xb = x.astype(BF)  # one contiguous cast, then per-core strided gathers
    in_maps = []
    for i in range(8):
        xs = xb[:, :, :, :, :, 12 * i:12 * (i + 1)]
        # axes after reshape:
        # b0 c1 pt2 kt3 qx4 lx5 kx6 sx7 qyh8 qyl9 ly10 ky11 sy12 pz13 kz14 sz15
        xr = xs.reshape(4, 2, 2, 3, 10, 2, 3, 2, 2, 5, 2, 3, 2, 2, 3, 2)
        # -> [K(c,kt,kx,ky,kz)=162, chunk(qx,qyh)=20, slab(sx,sy,sz)=8,
        #     incol(qyl,pt,lx,ly,pz,b)=320]
        xp = xr.transpose(1, 3, 6, 11, 14, 4, 8, 7, 12, 15, 9, 2, 5, 10, 13, 0)
        xp = np.ascontiguousarray(xp).reshape(162, 20, 8 * 320)
        # conv4 weights: this core's qz=i slice -> [128c, 100q, 256o]
        w4q = np.ascontiguousarray(
            w4f[:, :, 0, :, :, i].transpose(1, 2, 3, 0).reshape(128, 100 * 256)
        ).astype(BF)
        m = dict(shared)
        m["xp1"] = np.ascontiguousarray(xp[:128])
        m["xp2"] = np.ascontiguousarray(xp[128:])
        m["w4"] = w4q
        in_maps.append(m)
    return in_maps


def kernel(**inputs):
    global LAST_EXEC_NS
    if "nc" not in _CACHE:
        _CACHE["nc"] = _build()
    nc = _CACHE["nc"]
    in_maps = _prep(**inputs)
    tr = bool(_CACHE.get("trace"))
    kw = {}
    if tr:
        import shutil
        shutil.rmtree("/tmp/ktrace", ignore_errors=True)
        import os as _os
        _os.makedirs("/tmp/ktrace", exist_ok=True)
        kw["tmpdir"] = "/tmp/ktrace"
    res = run_bass_kernel_spmd(nc, in_maps, core_ids=list(range(8)),
                               trace=tr, **kw)
    LAST_EXEC_NS = res.exec_time_ns
    out = res.results[0]["out"]           # [6, 4]
    return np.ascontiguousarray(out.T)    # [4, 6]



# revision 37
# speedup vs baseline: 1.0807x; 1.0134x over previous
"""Trainium2 Bass kernel for nn_Actor (4D strided Minkowski-style conv net + MLP head).

Sharding v2: Z-block sharding. Z=96 splits into exactly 8 conv2-aligned blocks
of 12, so core i processes x[..., 12i:12(i+1)] for ALL 4 batches and locally
produces h2 for its own conv4 q-slice (qz=i). That makes conv4 q-sharded with
zero cross-core exchange before it; a single 4KB AllReduce of the conv4
partials replaces both AllGathers of the batch-sharded scheme.

conv4 streams w4 (the 105MB tensor, och-paged per q) through the PE as the
stationary operand in 200 [128,128] loads while h2 columns move; w4 is
prefetched into SBUF during the x-streaming phase so the tail is compute-only.

All heavy tensors are cast to bf16 host-side (halves HBM traffic; fp32 PSUM
accumulation keeps rel err ~9e-3, tolerance 2e-2). All convs have
kernel == stride, so each conv is a patch matmul. Host-side numpy only
rearranges layout / folds BN into conv weights (pure weight preprocessing).

Per-chunk pipeline: each 320-patch chunk is one merged DMA per input slab
group; pool tree runs on DVE (128-row part) and GpSimd (34-row part); after
every qx-stripe (2 chunks) conv2 + conv4-accumulate run so only AllReduce+MLP
remain after the stream.
"""

import sys

sys.path.insert(0, "/opt/trn_rl_repo")

from contextlib import ExitStack

import ml_dtypes
import numpy as np

import concourse.bass as bass
import concourse.tile as tile
from concourse import bacc, mybir
from concourse.bass_utils import run_bass_kernel_spmd

EPS = 1e-5
F32 = mybir.dt.float32
BF16 = mybir.dt.bfloat16
AF = mybir.ActivationFunctionType
BF = ml_dtypes.bfloat16

PC = 320           # patch columns per stream chunk
NCH = 6400 // PC   # 20 chunks; 2 chunks == 1 qx stripe
NQX = 10           # qx stripes; stripe s covers q = 10s..10s+9

LAST_EXEC_NS = None
_CACHE = {}


def _build():
    nc = bacc.Bacc(
        "TRN2",
        target_bir_lowering=False,
        debug=False,
        enable_asserts=False,
        num_devices=8,
    )
    d = {}

    def din(name, shape, dt=BF16):
        d[name] = nc.dram_tensor(name, list(shape), dt, kind="ExternalInput").ap()
        return d[name]

    xp1 = din("xp1", (128, NCH, 8 * PC))
    xp2 = din("xp2", (34, NCH, 8 * PC))
    w1a = din("w1a", (128, 64))
    w1b = din("w1b", (34, 64))
    b1 = din("b1", (64, 1), F32)
    w2 = din("w2", (64, 16 * 128))
    b2 = din("b2", (128, 1), F32)
    w4 = din("w4", (128, 100 * 256))
    b4 = din("b4", (128, 2), F32)
    wl0 = din("wl0", (12, 512), F32)
    wl1 = din("wl1", (128, 512), F32)
    wl2 = din("wl2", (128, 512), F32)
    bl = din("bl", (128, 4), F32)
    wo = din("wo", (128, 24), F32)
    bo = din("bo", (6, 1), F32)
    jnt = din("jnt", (12, 4), F32)
    out = nc.dram_tensor("out", [6, 4], F32, kind="ExternalOutput").ap()

    with TileKernel(nc) as tk:
        tk.run(xp1, xp2, w1a, w1b, b1, w2, b2, w4, b4,
               wl0, wl1, wl2, bl, wo, bo, jnt, out)
    nc.compile()
    return nc


class TileKernel:
    def __init__(self, nc):
        self.nc = nc
        self.tc = tile.TileContext(nc)
        self.ctx = ExitStack()

    def __enter__(self):
        self.tc.__enter__()
        return self

    def __exit__(self, *a):
        self.ctx.close()
        return self.tc.__exit__(*a)

    def run(self, xp1, xp2, w1a, w1b, b1, w2, b2, w4, b4,
            wl0, wl1, wl2, bl, wo, bo, jnt, out):
        nc, tc, ctx = self.nc, self.tc, self.ctx
        sync = nc.sync

        const = ctx.enter_context(tc.tile_pool(name="const", bufs=1))
        stream = ctx.enter_context(tc.tile_pool(name="stream", bufs=5))
        pools = ctx.enter_context(tc.tile_pool(name="pools", bufs=4))
        big = ctx.enter_context(tc.tile_pool(name="big", bufs=1))
        ps1 = ctx.enter_context(tc.tile_pool(name="ps1", bufs=3, space="PSUM"))
        ps2 = ctx.enter_context(tc.tile_pool(name="ps2", bufs=2, space="PSUM"))
        ps4 = ctx.enter_context(tc.tile_pool(name="ps4", bufs=1, space="PSUM"))
        psl = ctx.enter_context(tc.tile_pool(name="psl", bufs=1, space="PSUM"))
        pso = ctx.enter_context(tc.tile_pool(name="pso", bufs=1, space="PSUM"))
        dram = ctx.enter_context(tc.tile_pool(name="dram", bufs=1, space="DRAM"))

        # ---- constants into SBUF ----
        w1a_t = const.tile([128, 64], BF16); sync.dma_start(w1a_t[:], w1a)
        w1b_t = const.tile([34, 64], BF16); sync.dma_start(w1b_t[:], w1b)
        b1_t = const.tile([64, 1], F32); sync.dma_start(b1_t[:], b1)
        w2_t = const.tile([64, 16 * 128], BF16); sync.dma_start(w2_t[:], w2)
        b2_t = const.tile([128, 1], F32); sync.dma_start(b2_t[:], b2)
        b4_t = const.tile([128, 2], F32); sync.dma_start(b4_t[:], b4)
        wl0_t = const.tile([12, 512], F32); sync.dma_start(wl0_t[:], wl0)
        wl1_t = const.tile([128, 512], F32); sync.dma_start(wl1_t[:], wl1)
        wl2_t = const.tile([128, 512], F32); sync.dma_start(wl2_t[:], wl2)
        bl_t = const.tile([128, 4], F32); sync.dma_start(bl_t[:], bl)
        wo_t = const.tile([128, 24], F32); sync.dma_start(wo_t[:], wo)
        bo_t = const.tile([6, 1], F32); sync.dma_start(bo_t[:], bo)
        jnt_t = const.tile([12, 4], F32); sync.dma_start(jnt_t[:], jnt)

        # w4: prefetched stripe-by-stripe during the stream phase (scalar
        # HWDGE ring, so it never queues behind the x-stream triggers)
        w4_t = const.tile([128, 100 * 256], BF16)
        SW = 10 * 256  # cols per stripe

        def w4_fetch(s):
            nc.scalar.dma_start(w4_t[:, s * SW:(s + 1) * SW],
                                w4[:, s * SW:(s + 1) * SW])

        w4_fetch(0)
        w4tv = w4_t[:].rearrange("p (q o) -> p q o", q=100)

        h1 = big.tile([64, 6400], BF16)
        # h1 col = qx*640 + qy*64 + pt*32 + lx*16 + ly*8 + lz*4 + b
        h1v = h1[:].rearrange(
            "p (qx qy pt lx ly lz b) -> p qx qy pt lx ly lz b",
            qx=10, qy=10, pt=2, lx=2, ly=2, lz=2)
        h2 = big.tile([128, 400], BF16)       # col = q*4 + b
        h2v = h2[:].rearrange("p (q b) -> p q b", b=4)
        p4 = ps4.tile([128, 8], F32)          # conv4 acc: col = 4*och_half + b

        # ---- streaming phase: pool + conv1 per chunk; conv2+conv4 per stripe ----
        for c in range(NCH):
            if c % 2 == 0 and c // 2 + 1 < NQX:
                w4_fetch(c // 2 + 1)
            t1 = stream.tile([128, 8 * PC], BF16, tag="t1")
            sync.dma_start(t1[:], xp1[:, c, :])
            t2 = stream.tile([34, 8 * PC], BF16, tag="t2")
            nc.scalar.dma_start(t2[:], xp2[:, c, :])

            # t2's small DMA lands first: pool it while t1 is still in flight
            po2 = self._pooltree(pools, t2, 34, "b")
            po1 = self._pooltree(pools, t1, 128, "a")

            p1 = ps1.tile([64, PC], F32)
            nc.tensor.matmul(p1[:], w1a_t[:], po1[:], start=True, stop=False)
            nc.tensor.matmul(p1[:], w1b_t[:], po2[:], start=False, stop=True)
            nc.scalar.activation(h1[:, c * PC:(c + 1) * PC], p1[:],
                                 AF.Relu, bias=b1_t[:, 0:1])

            if c % 2 == 1:
                qx = c // 2
                # conv2 stripe: 16 shifted matmuls -> [128, 40] (qy, b)
                p2 = ps2.tile([128, 40], F32)
                li = 0
                for lt in range(2):
                    for lx in range(2):
                        for ly in range(2):
                            for lz in range(2):
                                rhs = h1v[:, qx, :, lt, lx, ly, lz, :]
                                nc.tensor.matmul(
                                    p2[:], w2_t[:, li * 128:(li + 1) * 128], rhs,
                                    start=(li == 0), stop=(li == 15))
                                li += 1
                nc.scalar.activation(h2[:, qx * 40:(qx + 1) * 40], p2[:],
                                     AF.Relu, bias=b2_t[:, 0:1])
                # conv4 stripe: accumulate q = 10*qx .. 10*qx+9
                for j in range(10):
                    q = qx * 10 + j
                    rhs4 = h2v[:, q, :]
                    for hh in range(2):
                        # NB: start=True resets the accumulate state of the
                        # whole 2KB PSUM zero region, so only the very first
                        # matmul of the group may set it.
                        nc.tensor.matmul(
                            p4[:, 4 * hh:4 * hh + 4],
                            w4tv[:, q, 128 * hh:128 * hh + 128], rhs4,
                            start=(q == 0 and hh == 0), stop=(q == 99 and hh == 1))

        # ---- AllReduce conv4 partials: [128, 8] f32 = 4KB ----
        po4 = big.tile([128, 8], F32, tag="po4")
        nc.scalar.activation(po4[:], p4[:], AF.Copy)
        ar_in = dram.tile([128, 8], F32, tag="ari")
        ar_out = dram.tile([128, 8], F32, tag="aro")
        sync.dma_start(ar_in[:], po4[:])
        nc.gpsimd.collective_compute(
            "AllReduce", mybir.AluOpType.add,
            replica_groups=[list(range(8))],
            ins=[ar_in[:].opt()], outs=[ar_out[:].opt()])
        h256 = big.tile([128, 8], F32, tag="h256")
        sync.dma_start(h256[:], ar_out[:])

        # bias + relu per och half: col = 4*half + b
        h256r = big.tile([128, 8], F32, tag="h256r")
        nc.scalar.activation(h256r[:, 0:4], h256[:, 0:4], AF.Relu, bias=b4_t[:, 0:1])
        nc.scalar.activation(h256r[:, 4:8], h256[:, 4:8], AF.Relu, bias=b4_t[:, 1:2])

        # ---- MLP head (redundant on every core) ----
        pl = psl.tile([128, 16], F32)
        for s in range(4):
            msl = slice(128 * s, 128 * (s + 1))
            pls = pl[:, 4 * s:4 * s + 4]
            nc.tensor.matmul(pls, wl0_t[:, msl], jnt_t[:], start=True, stop=False)
            nc.tensor.matmul(pls, wl1_t[:, msl], h256r[:, 0:4], start=False, stop=False)
            nc.tensor.matmul(pls, wl2_t[:, msl], h256r[:, 4:8], start=False, stop=True)
        hl = big.tile([128, 16], F32, tag="hl")
        for s in range(4):
            nc.scalar.activation(hl[:, 4 * s:4 * s + 4], pl[:, 4 * s:4 * s + 4],
                                 AF.Relu, bias=bl_t[:, s:s + 1])
        po = pso.tile([6, 4], F32)
        wo_v = wo_t[:].rearrange("p (s n) -> p s n", s=4)
        for s in range(4):
            nc.tensor.matmul(po[:], wo_v[:, s, :], hl[:, 4 * s:4 * s + 4],
                             start=(s == 0), stop=(s == 3))
        ot = big.tile([6, 4], F32, tag="ot")
        nc.scalar.activation(ot[:], po[:], AF.Tanh, bias=bo_t[:, 0:1])
        sync.dma_start(out, ot[:])

    def _pooltree(self, pool, t, p, tag):
        """max over the 8 pooled-window slabs: [p, 8, PC] -> [p, PC]."""
        nc = self.nc
        v8 = t[:].rearrange("p (x c f) -> p x c f", x=4, c=2)
        m4 = pool.tile([p, 4 * PC], BF16, tag=tag + "4")
        m4v = m4[:].rearrange("p (x f) -> p x f", x=4)
        nc.vector.tensor_tensor(m4v, v8[:, :, 0, :], v8[:, :, 1, :],
                                op=mybir.AluOpType.max)
        v4 = m4[:].rearrange("p (x c f) -> p x c f", x=2, c=2)
        m2 = pool.tile([p, 2 * PC], BF16, tag=tag + "2")
        m2v = m2[:].rearrange("p (x f) -> p x f", x=2)
        nc.vector.tensor_tensor(m2v, v4[:, :, 0, :], v4[:, :, 1, :],
                                op=mybir.AluOpType.max)
        v2 = m2[:].rearrange("p (c f) -> p c f", c=2)
        m1 = pool.tile([p, PC], BF16, tag=tag + "1")
        nc.vector.tensor_tensor(m1[:], v2[:, 0, :], v2[:, 1, :],
                                op=mybir.AluOpType.max)
        return m1



def _prep(x, jnt_pos, jnt_goal, w1, b1, g1, be1, m1, v1, w2, b2, g2, be2, m2, v2,
          w4, b4, gn, ben, mn, vn, wl, bl, gl, bel, ml, vl, wo, bo):
    """Host-side shard + layout prep (layout/precision only). 8 in_maps."""
    f = np.float32

    def fold(w, b, g, be, m, v):
        s = (g / np.sqrt(v + EPS)).astype(f)
        return (w * s.reshape((-1,) + (1,) * (w.ndim - 1))).astype(f), \
               ((b - m) * s + be).astype(f)

    w1f, b1f = fold(w1, b1, g1, be1, m1, v1)
    w2f, b2f = fold(w2, b2, g2, be2, m2, v2)
    w4f, b4f = fold(w4, b4, gn, ben, mn, vn)
    wlf, blf = fold(wl, bl, gl, bel, ml, vl)

    # conv1 lhsT [162, 64], K order (c, kt, kx, ky, kz)
    w1k = np.ascontiguousarray(
        w1f.transpose(1, 2, 3, 4, 5, 0).reshape(162, 64)).astype(BF)
    # conv2 lhsT per shift: [64, 16*128], shift order (lt, lx, ly, lz)
    w2k = np.ascontiguousarray(
        w2f.transpose(2, 3, 4, 5, 1, 0).reshape(16, 64, 128)
        .transpose(1, 0, 2).reshape(64, 16 * 128)).astype(BF)
    # MLP
    wlT = np.ascontiguousarray(wlf.T)          # [268, 512]
    wl0 = np.ascontiguousarray(wlT[0:12])
    wl1 = np.ascontiguousarray(wlT[12:140])
    wl2 = np.ascontiguousarray(wlT[140:268])
    bl4 = np.ascontiguousarray(blf.reshape(4, 128).T)     # [128, 4]
    wo4 = np.ascontiguousarray(
        wo.T.reshape(4, 128, 6).transpose(1, 0, 2).reshape(128, 24).astype(f))
    bo1 = np.ascontiguousarray(bo.reshape(6, 1).astype(f))
    b4r = np.ascontiguousarray(b4f.reshape(2, 128).T)     # [128, 2]
    jntc = np.ascontiguousarray(
        np.concatenate([jnt_pos, jnt_goal], axis=1).T.astype(f))  # [12, 4]

    shared = dict(
        w1a=np.ascontiguousarray(w1k[:128]),
        w1b=np.ascontiguousarray(w1k[128:]),
        b1=b1f.reshape(64, 1), w2=w2k, b2=b2f.reshape(128, 1),
        b4=b4r, wl0=wl0, wl1=wl1, wl2=wl2, bl=bl4, wo=wo4, bo=bo1, jnt=jntc,
    )

    xb = x.astype(BF)  # one contiguous cast, then per-core strided gathers
    in_maps = []
    for i in range(8):
        xs = xb[:, :, :, :, :, 12 * i:12 * (i + 1)]
        # axes after reshape:
        # b0 c1 pt2 kt3 qx4 lx5 kx6 sx7 qyh8 qyl9 ly10 ky11 sy12 pz13 kz14 sz15
        xr = xs.reshape(4, 2, 2, 3, 10, 2, 3, 2, 2, 5, 2, 3, 2, 2, 3, 2)
        # -> [K(c,kt,kx,ky,kz)=162, chunk(qx,qyh)=20, slab(sx,sy,sz)=8,
        #     incol(qyl,pt,lx,ly,pz,b)=320]
        xp = xr.transpose(1, 3, 6, 11, 14, 4, 8, 7, 12, 15, 9, 2, 5, 10, 13, 0)
        xp = np.ascontiguousarray(xp).reshape(162, 20, 8 * 320)
        # conv4 weights: this core's qz=i slice -> [128c, 100q, 256o]
        w4q = np.ascontiguousarray(
            w4f[:, :, 0, :, :, i].transpose(1, 2, 3, 0).reshape(128, 100 * 256)
        ).astype(BF)
        m = dict(shared)
        m["xp1"] = np.ascontiguousarray(xp[:128])
        m["xp2"] = np.ascontiguousarray(xp[128:])
        m["w4"] = w4q
        in_maps.append(m)
    return in_maps


def kernel(**inputs):
    global LAST_EXEC_NS
    if "nc" not in _CACHE:
        _CACHE["nc"] = _build()
    nc = _CACHE["nc"]
    in_maps = _prep(**inputs)
    tr = bool(_CACHE.get("trace"))
    kw = {}
    if tr:
        import shutil
        shutil.rmtree("/tmp/ktrace", ignore_errors=True)
        import os as _os
        _os.makedirs("/tmp/ktrace", exist_ok=True)
        kw["tmpdir"] = "/tmp/ktrace"
    res = run_bass_kernel_spmd(nc, in_maps, core_ids=list(range(8)),
                               trace=tr, **kw)
    LAST_EXEC_NS = res.exec_time_ns
    out = res.results[0]["out"]           # [6, 4]
    return np.ascontiguousarray(out.T)    # [4, 6]

